# revision 28
# baseline (speedup 1.0000x reference)
"""Trainium2 Bass kernel for nn_DecomposingAttnProcessor_pad.

Computation (per reference):
  q = hs @ Wq, k = enc @ Wk, v = enc @ Wv        (per-head, dh=64, H=16)
  scores = (q @ k^T) / 8                          [BTC=8, H, S=2048, E=256]
  softmax over the COMPONENT axis (BTC = 4 components x 2 samples)
  entropy = sum_c w*log(w)                        [B=2, H, S, E]
  out = (w @ v) @ Wo + hs    (pad stream contributes zero: v_pad = 0)

Sharding (8 cores, no collectives): 2 sample-groups (b) x 4 query-token
blocks (si, 512 tokens each). Each core holds all 4 components of its
sample b, so the component softmax is fully on-device. Host only
transposes/casts/concatenates (layout prep), all FLOPs are on device.

Device layouts (T = transposed, d on partitions):
  hsT  [1024, 2048] bf16  cols = c*512 + s_local
  encT [1024, 1024] bf16  cols = c*256 + e
  QT/KT computed transposed [d_out, tok]; V natural [tok, d_out].
  scoresT [e, s] per (h, c) -> softmax across the 4 component tiles.
  out matmul: outT[dh, s] = V[e, dh]^T @ w[e, s];  final: out = OT^T@Wo + hs.

Softmax/entropy identities (scale 1/8 folded into the QT copy):
  Z = sum_c exp(s_c); w_c = exp(s_c) * rZ with rZ = exp(-ln Z)
  H = (sum_c exp(s_c)*s_c) * rZ - ln Z    [fp32 path for accuracy]
"""

import sys

import numpy as np

sys.path.insert(0, "/opt/trn_rl_repo")

import ml_dtypes  # noqa: E402

import concourse.bass as bass  # noqa: E402
import concourse.tile as tile  # noqa: E402
from concourse import bacc, mybir  # noqa: E402
from concourse.bass_utils import run_bass_kernel_spmd  # noqa: E402

BF16 = mybir.dt.bfloat16
F32 = mybir.dt.float32
F32R = mybir.dt.float32r
F16 = mybir.dt.float16
AF = mybir.ActivationFunctionType


def _pin_act_table_set():
    """Restrict walrus' activation-table choice to the one set that holds
    Exp, Ln AND Copy, so the kernel loads activation tables exactly once
    instead of thrashing between exp-only and ln-only sets (~1.5us/load)."""
    import concourse.bacc as bacc_mod
    import concourse.hw_specs as hw_specs_mod

    orig = hw_specs_mod.get_activation_tables
    keep = "natural_log_exp_and_others"

    def patched(arch):
        tabs = orig(arch)
        if keep not in tabs:
            return tabs
        return {n: (f if n == keep else set()) for n, f in tabs.items()}

    bacc_mod.get_activation_tables = patched


def _enable_ldw_opt():
    """Flip walrus' --enable-ldw-opt to true: dedups/merges LDWEIGHTS so
    back-to-back matmuls don't each pay a full weight-load."""
    import concourse.bass_utils as bu

    if getattr(bu.run_command, "_ldw_patched", False):
        return
    orig = bu.run_command

    def patched(cmd, *a, **k):
        cmd = [c.replace("--enable-ldw-opt=false", "--enable-ldw-opt=true")
               if isinstance(c, str) else c for c in cmd]
        return orig(cmd, *a, **k)

    patched._ldw_patched = True
    bu.run_command = patched

NC_COMP = 4  # components
B = 2  # samples
H = 16  # heads
D = 1024  # hidden
S = 2048  # query tokens (full)
E = 256  # encoder tokens per btc
S_CORE = 512  # query tokens per core (S / 4 s-shards)
TOK = NC_COMP * S_CORE  # 2048 token-rows per core
ETOK = NC_COMP * E  # 1024 encoder token-rows per core
SCALE = 1.0 / 8.0  # 1/sqrt(dh)
KC = D // 128  # 8 contraction chunks

N_CORES = 8


def build_kernel(ctx, tc):
    nc = tc.nc

    ident_d = nc.dram_tensor("ident", [128, 128], F16, kind="ExternalInput").ap()
    hsT = nc.dram_tensor("hsT", [D, TOK], BF16, kind="ExternalInput").ap()
    encT = nc.dram_tensor("encT", [D, ETOK], BF16, kind="ExternalInput").ap()
    hs_res = nc.dram_tensor("hs_res", [TOK, D], F32, kind="ExternalInput").ap()
    wq_d = nc.dram_tensor("wq", [D, D], BF16, kind="ExternalInput").ap()
    wk_d = nc.dram_tensor("wk", [D, D], BF16, kind="ExternalInput").ap()
    wv_d = nc.dram_tensor("wv", [D, D], BF16, kind="ExternalInput").ap()
    wo_d = nc.dram_tensor("wo", [D, D], BF16, kind="ExternalInput").ap()
    out_d = nc.dram_tensor("out", [TOK, D], F32, kind="ExternalOutput").ap()
    # entropy layout: [h, e_chunk, e_in(128), s(512)]
    ent_d = nc.dram_tensor("ent", [H, 2, 128, S_CORE], F32, kind="ExternalOutput").ap()

    p_keep = ctx.enter_context(tc.tile_pool(name="keep", bufs=1))
    p_psum = ctx.enter_context(tc.tile_pool(name="psum", bufs=1, space="PSUM"))

    # --- resident SBUF tensors ---
    ident_sb = p_keep.tile([128, 128], F16)
    nc.sync.dma_start(ident_sb, ident_d)
    wq_sb = p_keep.tile([128, KC * D], BF16)  # [k-in-chunk, kc*1024 + dout]
    hsT_sb = p_keep.tile([128, KC * TOK], BF16)  # [k, kc*2048 + tok]
    kt_sb = p_keep.tile([128, KC * ETOK], BF16)  # [dout-in-chunk, t*1024 + tok]
    v_sb = p_keep.tile([128, KC * D], F16)  # [tok-in-chunk, tt*1024 + dout]
    ot_sb = p_keep.tile([128, KC * TOK], BF16)  # [dout-in-chunk, t*2048 + tok]

    # --- phase 1: KT and V projections (weights freed afterwards) ---
    with tc.tile_pool(name="ph1", bufs=1) as p1:
        wk_sb = p1.tile([128, KC * D], BF16)
        wv_sb = p1.tile([128, KC * D], BF16)
        encT_sb = p1.tile([128, KC * ETOK], BF16)
        # phase-1 inputs first so PE work starts ASAP; bulk hsT after
        for kc in range(KC):
            nc.sync.dma_start(wk_sb[:, bass.ts(kc, D)], wk_d[bass.ts(kc, 128), :])
            nc.sync.dma_start(wv_sb[:, bass.ts(kc, D)], wv_d[bass.ts(kc, 128), :])
            nc.sync.dma_start(encT_sb[:, bass.ts(kc, ETOK)], encT[bass.ts(kc, 128), :])
        for kc in range(KC):
            nc.sync.dma_start(wq_sb[:, bass.ts(kc, D)], wq_d[bass.ts(kc, 128), :])
            nc.sync.dma_start(hsT_sb[:, bass.ts(kc, TOK)], hsT[bass.ts(kc, 128), :])

        # KT[dout, tok]: lhsT = Wk[k, dout-chunk], rhs = encT[k, tok-chunk]
        for t in range(KC):
            for n in range(ETOK // 512):
                ps = p_psum.tile([128, 512], F32, tag="proj", bufs=2)
                for kc in range(KC):
                    nc.tensor.matmul(
                        ps,
                        wk_sb[:, kc * D + t * 128 : kc * D + (t + 1) * 128],
                        encT_sb[:, kc * ETOK + n * 512 : kc * ETOK + (n + 1) * 512],
                        start=(kc == 0),
                        stop=(kc == KC - 1),
                    )
                nc.scalar.activation(
                    kt_sb[:, t * ETOK + n * 512 : t * ETOK + (n + 1) * 512], ps, AF.Copy
                )
        # V[tok, dout]: lhsT = encT[k, tok-chunk], rhs = Wv[k, dout-chunk]
        for tt in range(KC):
            for n in range(D // 512):
                ps = p_psum.tile([128, 512], F32, tag="proj", bufs=2)
                for kc in range(KC):
                    nc.tensor.matmul(
                        ps,
                        encT_sb[:, kc * ETOK + tt * 128 : kc * ETOK + (tt + 1) * 128],
                        wv_sb[:, kc * D + n * 512 : kc * D + (n + 1) * 512],
                        start=(kc == 0),
                        stop=(kc == KC - 1),
                    )
                nc.scalar.activation(
                    v_sb[:, tt * D + n * 512 : tt * D + (n + 1) * 512], ps, AF.Copy
                )

    # --- phase 2: per dout-chunk t (= head pair 2t, 2t+1) ---
    with tc.tile_pool(name="ph2", bufs=1) as p2:
        qts = {}

        def emit_qt_part(t, ns, qt):
            # QT tile [128 dout, TOK], scaled by 1/8 on copy-out
            for n in ns:
                ps = p_psum.tile([128, 512], F32, tag="proj", bufs=2, name=f"qtps{t}_{n}")
                for kc in range(KC):
                    nc.tensor.matmul(
                        ps,
                        wq_sb[:, kc * D + t * 128 : kc * D + (t + 1) * 128],
                        hsT_sb[:, kc * TOK + n * 512 : kc * TOK + (n + 1) * 512],
                        start=(kc == 0),
                        stop=(kc == KC - 1),
                    )
                nc.scalar.activation(qt[:, bass.ts(n, 512)], ps, AF.Copy, scale=SCALE)

        def new_qt(t):
            qts[t] = p2.tile([128, TOK], BF16, tag="qt", bufs=2, name=f"qt{t}")
            return qts[t]

        def head_pair(t):
            qt = qts[t]
            e_t = [{}, {}]
            es_t = [{}, {}]
            z_pss = [{}, {}]
            w_t = [{}, {}]
            lnzs = [None, None]
            # phase A for both heads: 16 dense score matmuls on PE
            for hh in range(2):
                hp = slice(hh * 64, (hh + 1) * 64)
                for ec in range(2):
                    for c in range(NC_COMP):
                        sc_ps = p_psum.tile([128, 512], F32, tag="sc", bufs=4)
                        nc.tensor.matmul(
                            sc_ps,
                            kt_sb[
                                hp,
                                t * ETOK + c * 256 + ec * 128 : t * ETOK + c * 256 + (ec + 1) * 128,
                            ],
                            qt[hp, bass.ts(c, 512)],
                            start=True,
                            stop=True,
                        )
                        e_sb = p2.tile([128, 512], F16, tag="e", bufs=16)
                        nc.scalar.activation(e_sb, sc_ps, AF.Exp)
                        es_sb = p2.tile([128, 512], F16, tag="es", bufs=12)
                        nc.vector.tensor_mul(es_sb, e_sb, sc_ps)
                        e_t[hh][(c, ec)] = e_sb
                        es_t[hh][(c, ec)] = es_sb

            # phase B both heads: Z via identity-matmul accumulation on PE
            for hh in range(2):
                for ec in range(2):
                    z_ps = p_psum.tile([128, 512], F32, tag="zo", bufs=2)
                    for c in range(NC_COMP):
                        nc.tensor.matmul(
                            z_ps,
                            ident_sb,
                            e_t[hh][(c, ec)],
                            start=(c == 0),
                            stop=(c == NC_COMP - 1),
                        )
                    z_pss[hh][ec] = z_ps

            # phase C both heads: softmax scalars, w before entropy tail
            for hh in range(2):
                h = 2 * t + hh
                lnz = p2.tile([128, 2 * 512], F32, tag="lnz", bufs=2)
                lnzs[hh] = lnz
                for ec in range(2):
                    nc.scalar.activation(
                        lnz[:, bass.ts(ec, 512)], z_pss[hh][ec], AF.Ln
                    )
                    rz = p2.tile([128, 512], F16, tag="rz", bufs=4)
                    nc.scalar.activation(
                        rz, lnz[:, bass.ts(ec, 512)], AF.Exp, scale=-1.0
                    )
                    for c in range(NC_COMP):
                        w_sb = p2.tile([128, 512], F16, tag="w", bufs=16)
                        nc.vector.tensor_mul(w_sb, e_t[hh][(c, ec)], rz)
                        w_t[hh][(c, ec)] = w_sb
                    t01 = p2.tile([128, 512], F16, tag="tz", bufs=4)
                    nc.gpsimd.tensor_add(t01, es_t[hh][(0, ec)], es_t[hh][(1, ec)])
                    t23 = p2.tile([128, 512], F16, tag="tz", bufs=4)
                    nc.gpsimd.tensor_add(t23, es_t[hh][(2, ec)], es_t[hh][(3, ec)])
                    tt_ = p2.tile([128, 512], F16, tag="tz", bufs=4)
                    nc.gpsimd.tensor_add(tt_, t01, t23)
                    uu = p2.tile([128, 512], F16, tag="tz", bufs=4)
                    nc.vector.tensor_mul(uu, tt_, rz)
                    hent = p2.tile([128, 512], F32, tag="hent", bufs=2)
                    nc.vector.tensor_sub(hent, uu, lnz[:, bass.ts(ec, 512)])
                    nc.sync.dma_start(ent_d[h, ec], hent)

            # phase D: next head-pair's QT matmuls fill the PE window
            if t + 1 < KC:
                emit_qt_part(t + 1, (0, 1, 2, 3), new_qt(t + 1))

            # phase E both heads: outT[dh, s]; 2 components per psum bank
            for hh in range(2):
                h = 2 * t + hh
                hp = slice(hh * 64, (hh + 1) * 64)
                out_pss = []
                for cp in range(2):
                    out_ps = p_psum.tile([128, 512], F32, tag="zo", bufs=2)
                    out_pss.append(out_ps)
                    for ci in range(2):
                        c = 2 * cp + ci
                        for ec in range(2):
                            nc.tensor.matmul(
                                out_ps[ci * 64 : (ci + 1) * 64, :],
                                v_sb[
                                    :,
                                    (2 * c + ec) * D + h * 64 : (2 * c + ec) * D + h * 64 + 64,
                                ],
                                w_t[hh][(c, ec)],
                                start=(ec == 0),
                                stop=(ec == 1),
                            )
                for cp in range(2):
                    for ci in range(2):
                        c = 2 * cp + ci
                        nc.scalar.activation(
                            ot_sb[hp, t * TOK + c * 512 : t * TOK + (c + 1) * 512],
                            out_pss[cp][ci * 64 : (ci + 1) * 64, :],
                            AF.Copy,
                        )

        emit_qt_part(0, (0, 1, 2, 3), new_qt(0))
        for t in range(KC):
            head_pair(t)
            del qts[t]

    # --- phase 3: out = OT^T @ Wo + residual ---
    with tc.tile_pool(name="ph3", bufs=1) as p3:
        wo_sb = p3.tile([128, KC * D], BF16)
        for kc in range(KC):
            nc.sync.dma_start(wo_sb[:, bass.ts(kc, D)], wo_d[bass.ts(kc, 128), :])
        for m in range(TOK // 128):
            res = p3.tile([128, D], F32, tag="res", bufs=2)
            nc.sync.dma_start(res, hs_res[bass.ts(m, 128), :])
            o_sb = p3.tile([128, D], F32, tag="osb", bufs=3)
            for n in range(D // 512):
                ps = p_psum.tile([128, 512], F32, tag="proj", bufs=2)
                for kc in range(KC):
                    nc.tensor.matmul(
                        ps,
                        ot_sb[:, kc * TOK + m * 128 : kc * TOK + (m + 1) * 128],
                        wo_sb[:, kc * D + n * 512 : kc * D + (n + 1) * 512],
                        start=(kc == 0),
                        stop=(kc == KC - 1),
                    )
                nc.vector.tensor_add(
                    o_sb[:, bass.ts(n, 512)], ps, res[:, bass.ts(n, 512)]
                )
            nc.sync.dma_start(out_d[bass.ts(m, 128), :], o_sb)


_CACHE = {}


def _get_compiled():
    if "nc" not in _CACHE:
        from contextlib import ExitStack

        _pin_act_table_set()
        nc = bacc.Bacc("TRN2", target_bir_lowering=False, debug=False)
        with tile.TileContext(nc) as tc:
            with ExitStack() as ctx:
                build_kernel(ctx, tc)
        nc.compile()
        _CACHE["nc"] = nc
    return _CACHE["nc"]


def _prep_inputs(hidden_states, encoder_hidden_states, Wq, Wk, Wv, Wo):
    bf = ml_dtypes.bfloat16
    hs4 = np.ascontiguousarray(hidden_states.reshape(NC_COMP, B, S, D))
    enc4 = np.ascontiguousarray(encoder_hidden_states.reshape(NC_COMP, B, E, D))
    ws = {
        "ident": np.eye(128, dtype=np.float16),
        "wq": np.ascontiguousarray(Wq.astype(bf)),
        "wk": np.ascontiguousarray(Wk.astype(bf)),
        "wv": np.ascontiguousarray(Wv.astype(bf)),
        "wo": np.ascontiguousarray(Wo.astype(bf)),
    }
    in_maps = []
    for core in range(N_CORES):
        b, si = core // 4, core % 4
        sl = hs4[:, b, si * S_CORE : (si + 1) * S_CORE, :]  # [4, 512, 1024]
        hsT_a = np.ascontiguousarray(sl.transpose(2, 0, 1).reshape(D, TOK).astype(bf))
        hs_res_a = np.ascontiguousarray(sl.reshape(TOK, D).astype(np.float32))
        en = enc4[:, b]  # [4, 256, 1024]
        encT_a = np.ascontiguousarray(en.transpose(2, 0, 1).reshape(D, ETOK).astype(bf))
        in_maps.append({"hsT": hsT_a, "hs_res": hs_res_a, "encT": encT_a, **ws})
    return in_maps


def _assemble(results):
    out_full = np.empty((NC_COMP * B, S, D), np.float32)
    ent_full = np.empty((B, H, S, E), np.float32)
    for core in range(N_CORES):
        b, si = core // 4, core % 4
        o = results[core]["out"].reshape(NC_COMP, S_CORE, D)
        for c in range(NC_COMP):
            out_full[c * B + b, si * S_CORE : (si + 1) * S_CORE, :] = o[c]
        ent = results[core]["ent"].reshape(H, E, S_CORE)  # [h, e, s]
        ent_full[b, :, si * S_CORE : (si + 1) * S_CORE, :] = ent.transpose(0, 2, 1)
    return out_full, ent_full


def kernel(hidden_states, encoder_hidden_states, Wq, Wk, Wv, Wo, pad_length,
           **_run_kwargs):
    hidden_states = np.asarray(hidden_states)
    encoder_hidden_states = np.asarray(encoder_hidden_states)
    assert hidden_states.shape == (NC_COMP * B, S, D)
    assert encoder_hidden_states.shape == (NC_COMP * B, E, D)
    assert int(pad_length) == E

    nc = _get_compiled()
    in_maps = _prep_inputs(
        hidden_states, encoder_hidden_states,
        np.asarray(Wq), np.asarray(Wk), np.asarray(Wv), np.asarray(Wo),
    )
    res = run_bass_kernel_spmd(nc, in_maps, list(range(N_CORES)), **_run_kwargs)
    out_full, ent_full = _assemble(res.results)
    if not (np.isfinite(out_full).all() and np.isfinite(ent_full).all()):
        # transient device-state corruption observed once after a failed
        # NEFF load on the same core; a clean rerun recovers
        res = run_bass_kernel_spmd(nc, in_maps, list(range(N_CORES)), **_run_kwargs)
        out_full, ent_full = _assemble(res.results)
    if _run_kwargs:
        _CACHE["last_results"] = res
    return out_full, ent_full


# revision 29
# speedup vs baseline: 1.0322x; 1.0322x over previous
"""Trainium2 Bass kernel for nn_DecomposingAttnProcessor_pad.

Computation (per reference):
  q = hs @ Wq, k = enc @ Wk, v = enc @ Wv        (per-head, dh=64, H=16)
  scores = (q @ k^T) / 8                          [BTC=8, H, S=2048, E=256]
  softmax over the COMPONENT axis (BTC = 4 components x 2 samples)
  entropy = sum_c w*log(w)                        [B=2, H, S, E]
  out = (w @ v) @ Wo + hs    (pad stream contributes zero: v_pad = 0)

Sharding (8 cores, no collectives): 2 sample-groups (b) x 4 query-token
blocks (si, 512 tokens each). Each core holds all 4 components of its
sample b, so the component softmax is fully on-device. Host only
transposes/casts/concatenates (layout prep), all FLOPs are on device.

Device layouts (T = transposed, d on partitions):
  hsT  [1024, 2048] bf16  cols = c*512 + s_local
  encT [1024, 1024] bf16  cols = c*256 + e
  QT/KT computed transposed [d_out, tok]; V natural [tok, d_out].
  scoresT [e, s] per (h, c) -> softmax across the 4 component tiles.
  out matmul: outT[dh, s] = V[e, dh]^T @ w[e, s];  final: out = OT^T@Wo + hs.

Softmax/entropy identities (scale 1/8 folded into the QT copy):
  Z = sum_c exp(s_c); w_c = exp(s_c) * rZ with rZ = exp(-ln Z)
  H = (sum_c exp(s_c)*s_c) * rZ - ln Z    [fp32 path for accuracy]
"""

import sys

import numpy as np

sys.path.insert(0, "/opt/trn_rl_repo")

import ml_dtypes  # noqa: E402

import concourse.bass as bass  # noqa: E402
import concourse.tile as tile  # noqa: E402
from concourse import bacc, mybir  # noqa: E402
from concourse.bass_utils import run_bass_kernel_spmd  # noqa: E402

BF16 = mybir.dt.bfloat16
F32 = mybir.dt.float32
F32R = mybir.dt.float32r
F16 = mybir.dt.float16
AF = mybir.ActivationFunctionType


def _pin_act_table_set():
    """Restrict walrus' activation-table choice to the one set that holds
    Exp, Ln AND Copy, so the kernel loads activation tables exactly once
    instead of thrashing between exp-only and ln-only sets (~1.5us/load)."""
    import concourse.bacc as bacc_mod
    import concourse.hw_specs as hw_specs_mod

    orig = hw_specs_mod.get_activation_tables
    keep = "natural_log_exp_and_others"

    def patched(arch):
        tabs = orig(arch)
        if keep not in tabs:
            return tabs
        return {n: (f if n == keep else set()) for n, f in tabs.items()}

    bacc_mod.get_activation_tables = patched


def _enable_ldw_opt():
    """Flip walrus' --enable-ldw-opt to true: dedups/merges LDWEIGHTS so
    back-to-back matmuls don't each pay a full weight-load."""
    import concourse.bass_utils as bu

    if getattr(bu.run_command, "_ldw_patched", False):
        return
    orig = bu.run_command

    def patched(cmd, *a, **k):
        cmd = [c.replace("--enable-ldw-opt=false", "--enable-ldw-opt=true")
               if isinstance(c, str) else c for c in cmd]
        return orig(cmd, *a, **k)

    patched._ldw_patched = True
    bu.run_command = patched

NC_COMP = 4  # components
B = 2  # samples
H = 16  # heads
D = 1024  # hidden
S = 2048  # query tokens (full)
E = 256  # encoder tokens per btc
S_CORE = 512  # query tokens per core (S / 4 s-shards)
TOK = NC_COMP * S_CORE  # 2048 token-rows per core
ETOK = NC_COMP * E  # 1024 encoder token-rows per core
SCALE = 1.0 / 8.0  # 1/sqrt(dh)
KC = D // 128  # 8 contraction chunks

N_CORES = 8


def build_kernel(ctx, tc):
    nc = tc.nc

    ident_d = nc.dram_tensor("ident", [128, 128], F16, kind="ExternalInput").ap()
    hsT = nc.dram_tensor("hsT", [D, TOK], BF16, kind="ExternalInput").ap()
    encT = nc.dram_tensor("encT", [D, ETOK], BF16, kind="ExternalInput").ap()
    hs_res = nc.dram_tensor("hs_res", [TOK, D], F32, kind="ExternalInput").ap()
    wq_d = nc.dram_tensor("wq", [D, D], BF16, kind="ExternalInput").ap()
    wk_d = nc.dram_tensor("wk", [D, D], BF16, kind="ExternalInput").ap()
    wv_d = nc.dram_tensor("wv", [D, D], BF16, kind="ExternalInput").ap()
    wo_d = nc.dram_tensor("wo", [D, D], BF16, kind="ExternalInput").ap()
    out_d = nc.dram_tensor("out", [TOK, D], F32, kind="ExternalOutput").ap()
    # entropy layout: [h, e_chunk, e_in(128), s(512)]
    ent_d = nc.dram_tensor("ent", [H, 2, 128, S_CORE], F32, kind="ExternalOutput").ap()

    p_keep = ctx.enter_context(tc.tile_pool(name="keep", bufs=1))
    p_psum = ctx.enter_context(tc.tile_pool(name="psum", bufs=1, space="PSUM"))

    # --- resident SBUF tensors ---
    ident_sb = p_keep.tile([128, 128], F16)
    nc.sync.dma_start(ident_sb, ident_d)
    wq_sb = p_keep.tile([128, KC * D], BF16)  # [k-in-chunk, kc*1024 + dout]
    hsT_sb = p_keep.tile([128, KC * TOK], BF16)  # [k, kc*2048 + tok]
    kt_sb = p_keep.tile([128, KC * ETOK], BF16)  # [dout-in-chunk, t*1024 + tok]
    v_sb = p_keep.tile([128, KC * D], F16)  # [tok-in-chunk, tt*1024 + dout]
    ot_sb = p_keep.tile([128, KC * TOK], BF16)  # [dout-in-chunk, t*2048 + tok]

    # --- phase 1: KT and V projections (weights freed afterwards) ---
    with tc.tile_pool(name="ph1", bufs=1) as p1:
        wk_sb = p1.tile([128, KC * D], BF16)
        wv_sb = p1.tile([128, KC * D], BF16)
        encT_sb = p1.tile([128, KC * ETOK], BF16)
        # phase-1 inputs first so PE work starts ASAP; bulk hsT after
        for kc in range(KC):
            nc.sync.dma_start(wk_sb[:, bass.ts(kc, D)], wk_d[bass.ts(kc, 128), :])
            nc.sync.dma_start(wv_sb[:, bass.ts(kc, D)], wv_d[bass.ts(kc, 128), :])
            nc.sync.dma_start(encT_sb[:, bass.ts(kc, ETOK)], encT[bass.ts(kc, 128), :])
        for kc in range(KC):
            nc.sync.dma_start(wq_sb[:, bass.ts(kc, D)], wq_d[bass.ts(kc, 128), :])
            nc.sync.dma_start(hsT_sb[:, bass.ts(kc, TOK)], hsT[bass.ts(kc, 128), :])

        # KT[dout, tok]: lhsT = Wk[k, dout-chunk], rhs = encT[k, tok-chunk]
        for t in range(KC):
            for n in range(ETOK // 512):
                ps = p_psum.tile([128, 512], F32, tag="proj", bufs=2)
                for kc in range(KC):
                    nc.tensor.matmul(
                        ps,
                        wk_sb[:, kc * D + t * 128 : kc * D + (t + 1) * 128],
                        encT_sb[:, kc * ETOK + n * 512 : kc * ETOK + (n + 1) * 512],
                        start=(kc == 0),
                        stop=(kc == KC - 1),
                    )
                nc.scalar.activation(
                    kt_sb[:, t * ETOK + n * 512 : t * ETOK + (n + 1) * 512], ps, AF.Copy
                )
        # V[tok, dout]: lhsT = encT[k, tok-chunk], rhs = Wv[k, dout-chunk]
        for tt in range(KC):
            for n in range(D // 512):
                ps = p_psum.tile([128, 512], F32, tag="proj", bufs=2)
                for kc in range(KC):
                    nc.tensor.matmul(
                        ps,
                        encT_sb[:, kc * ETOK + tt * 128 : kc * ETOK + (tt + 1) * 128],
                        wv_sb[:, kc * D + n * 512 : kc * D + (n + 1) * 512],
                        start=(kc == 0),
                        stop=(kc == KC - 1),
                    )
                nc.scalar.activation(
                    v_sb[:, tt * D + n * 512 : tt * D + (n + 1) * 512], ps, AF.Copy
                )

    # --- phase 2: per dout-chunk t (= head pair 2t, 2t+1) ---
    with tc.tile_pool(name="ph2", bufs=1) as p2:
        qts = {}

        def emit_qt_part(t, ns, qt):
            # QT tile [128 dout, TOK], scaled by 1/8 on copy-out
            for n in ns:
                ps = p_psum.tile([128, 512], F32, tag="proj", bufs=2, name=f"qtps{t}_{n}")
                for kc in range(KC):
                    nc.tensor.matmul(
                        ps,
                        wq_sb[:, kc * D + t * 128 : kc * D + (t + 1) * 128],
                        hsT_sb[:, kc * TOK + n * 512 : kc * TOK + (n + 1) * 512],
                        start=(kc == 0),
                        stop=(kc == KC - 1),
                    )
                nc.scalar.activation(qt[:, bass.ts(n, 512)], ps, AF.Copy, scale=SCALE)

        def new_qt(t):
            qts[t] = p2.tile([128, TOK], BF16, tag="qt", bufs=2, name=f"qt{t}")
            return qts[t]

        def head_block(t, hh):
            qt = qts[t]
            h = 2 * t + hh
            hp = slice(hh * 64, (hh + 1) * 64)
            # phase A: all 8 score matmuls + exp + exp*score
            e_t = {}
            es_t = {}
            for ec in range(2):
                for c in range(NC_COMP):
                    sc_ps = p_psum.tile([128, 512], F32, tag="sc", bufs=4)
                    nc.tensor.matmul(
                        sc_ps,
                        kt_sb[
                            hp,
                            t * ETOK + c * 256 + ec * 128 : t * ETOK + c * 256 + (ec + 1) * 128,
                        ],
                        qt[hp, bass.ts(c, 512)],
                        start=True,
                        stop=True,
                    )
                    e_sb = p2.tile([128, 512], F16, tag="e", bufs=9)
                    nc.scalar.activation(e_sb, sc_ps, AF.Exp)
                    es_sb = p2.tile([128, 512], F32, tag="es", bufs=9)
                    nc.vector.tensor_mul(es_sb, e_sb, sc_ps)
                    e_t[(c, ec)] = e_sb
                    es_t[(c, ec)] = es_sb

            # phase B: Z = sum_c exp(s_c) via identity-matmul accumulation on PE
            z_pss = {}
            for ec in range(2):
                z_ps = p_psum.tile([128, 512], F32, tag="zo", bufs=2)
                for c in range(NC_COMP):
                    nc.tensor.matmul(
                        z_ps,
                        ident_sb,
                        e_t[(c, ec)],
                        start=(c == 0),
                        stop=(c == NC_COMP - 1),
                    )
                z_pss[ec] = z_ps

            # phase C: softmax scalars; w (DVE) BEFORE the entropy tail so the
            # out matmuls unblock as early as possible
            w_t = {}
            for ec in range(2):
                lnz = p2.tile([128, 512], F32, tag="lnz", bufs=3)
                nc.scalar.activation(lnz, z_pss[ec], AF.Ln)
                rz = p2.tile([128, 512], F16, tag="rz", bufs=3)
                nc.scalar.activation(rz, lnz, AF.Exp, scale=-1.0)
                for c in range(NC_COMP):
                    w_sb = p2.tile([128, 512], F16, tag="w", bufs=10)
                    nc.vector.tensor_mul(w_sb, e_t[(c, ec)], rz)
                    w_t[(c, ec)] = w_sb
                # T = sum_c exp(s_c)*s_c on POOL; entropy tail on DVE
                t01 = p2.tile([128, 512], F32, tag="tz", bufs=4)
                nc.gpsimd.tensor_add(t01, es_t[(0, ec)], es_t[(1, ec)])
                t23 = p2.tile([128, 512], F32, tag="tz", bufs=4)
                nc.gpsimd.tensor_add(t23, es_t[(2, ec)], es_t[(3, ec)])
                tt_ = p2.tile([128, 512], F32, tag="tz", bufs=4)
                nc.gpsimd.tensor_add(tt_, t01, t23)
                uu = p2.tile([128, 512], F32, tag="tz", bufs=4)
                nc.vector.tensor_mul(uu, tt_, rz)
                hent = p2.tile([128, 512], F32, tag="hent", bufs=2)
                nc.vector.tensor_sub(hent, uu, lnz)
                nc.sync.dma_start(ent_d[h, ec], hent)

            # phase D: next head-pair's QT matmuls fill the PE while DVE
            # finishes the w tiles
            if hh == 0:
                if t + 1 < KC:
                    emit_qt_part(t + 1, (0, 1), new_qt(t + 1))
            else:
                if t + 1 < KC:
                    emit_qt_part(t + 1, (2, 3), qts[t + 1])

            # phase E: outT[dh, s]; pack 2 components per psum bank
            out_pss = []
            for cp in range(2):
                out_ps = p_psum.tile([128, 512], F32, tag="zo", bufs=2)
                out_pss.append(out_ps)
                for ci in range(2):
                    c = 2 * cp + ci
                    for ec in range(2):
                        nc.tensor.matmul(
                            out_ps[ci * 64 : (ci + 1) * 64, :],
                            v_sb[
                                :,
                                (2 * c + ec) * D + h * 64 : (2 * c + ec) * D + h * 64 + 64,
                            ],
                            w_t[(c, ec)],
                            start=(ec == 0),
                            stop=(ec == 1),
                        )
            for cp in range(2):
                for ci in range(2):
                    c = 2 * cp + ci
                    nc.scalar.activation(
                        ot_sb[hp, t * TOK + c * 512 : t * TOK + (c + 1) * 512],
                        out_pss[cp][ci * 64 : (ci + 1) * 64, :],
                        AF.Copy,
                    )

        emit_qt_part(0, (0, 1, 2, 3), new_qt(0))
        for t in range(KC):
            head_block(t, 0)
            head_block(t, 1)
            del qts[t]

    # --- phase 3: out = OT^T @ Wo + residual ---
    with tc.tile_pool(name="ph3", bufs=1) as p3:
        wo_sb = p3.tile([128, KC * D], BF16)
        for kc in range(KC):
            nc.sync.dma_start(wo_sb[:, bass.ts(kc, D)], wo_d[bass.ts(kc, 128), :])
        for m in range(TOK // 128):
            res = p3.tile([128, D], F32, tag="res", bufs=2)
            nc.sync.dma_start(res, hs_res[bass.ts(m, 128), :])
            o_sb = p3.tile([128, D], F32, tag="osb", bufs=3)
            for n in range(D // 512):
                ps = p_psum.tile([128, 512], F32, tag="proj", bufs=2)
                for kc in range(KC):
                    nc.tensor.matmul(
                        ps,
                        ot_sb[:, kc * TOK + m * 128 : kc * TOK + (m + 1) * 128],
                        wo_sb[:, kc * D + n * 512 : kc * D + (n + 1) * 512],
                        start=(kc == 0),
                        stop=(kc == KC - 1),
                    )
                nc.vector.tensor_add(
                    o_sb[:, bass.ts(n, 512)], ps, res[:, bass.ts(n, 512)]
                )
            nc.sync.dma_start(out_d[bass.ts(m, 128), :], o_sb)


_CACHE = {}


def _get_compiled():
    if "nc" not in _CACHE:
        from contextlib import ExitStack

        _pin_act_table_set()
        nc = bacc.Bacc("TRN2", target_bir_lowering=False, debug=False)
        with tile.TileContext(nc) as tc:
            with ExitStack() as ctx:
                build_kernel(ctx, tc)
        nc.compile()
        _CACHE["nc"] = nc
    return _CACHE["nc"]


def _prep_inputs(hidden_states, encoder_hidden_states, Wq, Wk, Wv, Wo):
    bf = ml_dtypes.bfloat16
    hs4 = np.ascontiguousarray(hidden_states.reshape(NC_COMP, B, S, D))
    enc4 = np.ascontiguousarray(encoder_hidden_states.reshape(NC_COMP, B, E, D))
    ws = {
        "ident": np.eye(128, dtype=np.float16),
        "wq": np.ascontiguousarray(Wq.astype(bf)),
        "wk": np.ascontiguousarray(Wk.astype(bf)),
        "wv": np.ascontiguousarray(Wv.astype(bf)),
        "wo": np.ascontiguousarray(Wo.astype(bf)),
    }
    in_maps = []
    for core in range(N_CORES):
        b, si = core // 4, core % 4
        sl = hs4[:, b, si * S_CORE : (si + 1) * S_CORE, :]  # [4, 512, 1024]
        hsT_a = np.ascontiguousarray(sl.transpose(2, 0, 1).reshape(D, TOK).astype(bf))
        hs_res_a = np.ascontiguousarray(sl.reshape(TOK, D).astype(np.float32))
        en = enc4[:, b]  # [4, 256, 1024]
        encT_a = np.ascontiguousarray(en.transpose(2, 0, 1).reshape(D, ETOK).astype(bf))
        in_maps.append({"hsT": hsT_a, "hs_res": hs_res_a, "encT": encT_a, **ws})
    return in_maps


def _assemble(results):
    out_full = np.empty((NC_COMP * B, S, D), np.float32)
    ent_full = np.empty((B, H, S, E), np.float32)
    for core in range(N_CORES):
        b, si = core // 4, core % 4
        o = results[core]["out"].reshape(NC_COMP, S_CORE, D)
        for c in range(NC_COMP):
            out_full[c * B + b, si * S_CORE : (si + 1) * S_CORE, :] = o[c]
        ent = results[core]["ent"].reshape(H, E, S_CORE)  # [h, e, s]
        ent_full[b, :, si * S_CORE : (si + 1) * S_CORE, :] = ent.transpose(0, 2, 1)
    return out_full, ent_full


def kernel(hidden_states, encoder_hidden_states, Wq, Wk, Wv, Wo, pad_length,
           **_run_kwargs):
    hidden_states = np.asarray(hidden_states)
    encoder_hidden_states = np.asarray(encoder_hidden_states)
    assert hidden_states.shape == (NC_COMP * B, S, D)
    assert encoder_hidden_states.shape == (NC_COMP * B, E, D)
    assert int(pad_length) == E

    nc = _get_compiled()
    in_maps = _prep_inputs(
        hidden_states, encoder_hidden_states,
        np.asarray(Wq), np.asarray(Wk), np.asarray(Wv), np.asarray(Wo),
    )
    res = run_bass_kernel_spmd(nc, in_maps, list(range(N_CORES)), **_run_kwargs)
    out_full, ent_full = _assemble(res.results)
    if not (np.isfinite(out_full).all() and np.isfinite(ent_full).all()):
        # transient device-state corruption observed once after a failed
        # NEFF load on the same core; a clean rerun recovers
        res = run_bass_kernel_spmd(nc, in_maps, list(range(N_CORES)), **_run_kwargs)
        out_full, ent_full = _assemble(res.results)
    if _run_kwargs:
        _CACHE["last_results"] = res
    return out_full, ent_full
